# revision 4
# baseline (speedup 1.0000x reference)
"""Multi-head causal attention (B=4, T=2048, D=1024, H=16, hd=64) on 8 TRN2 cores.

Sharding: tensor-parallel over heads — 2 heads per core, all batches. Each core:
  - QKV projections for its 128 output dims (bf16 matmuls, fp32 PSUM accum)
  - scores computed TRANSPOSED (ST[k,q]) so no P transposes are needed
  - softmax without max-subtraction (scores bounded ~+-3); denominators come
    free from a ones-column appended to V; normalization via GpSimd
    partition-broadcast + elementwise multiply, deferred one q-chunk off the
    critical path
  - partial output projection against its 128 columns of Wo
Emission is software-pipelined across batches: while the ACT engine grinds
through exp() for batch b's attention, the PE stream is fed filler work from
batch b+1's QKV projection and batch b-1's output projection. All matmul
operands are bf16 (halves LDWEIGHTS time so loads hide under the previous
matmul's stream; fp32r pays ~4x on sub-256-wide tiles).
Host: pre-transpose/cast inputs to bf16, sum the 8 partial outputs, add bias.
"""
import numpy as np
import ml_dtypes

import concourse.bass as bass
import concourse.tile as tile
from concourse import bacc, mybir
from concourse.bass_utils import run_bass_kernel_spmd

F32 = mybir.dt.float32
BF16 = mybir.dt.bfloat16
EXP = mybir.ActivationFunctionType.Exp

B, T, D = 4, 2048, 1024
NCORES = 8
TT = B * T        # 8192 tokens
CT = D // 128     # 8 contraction tiles
NQ = T // 512     # 4 q-chunks per batch
NK = T // 128     # 16 k-tiles per batch
LOOK = 7          # score->PV software-pipeline lookahead


def build_nc():
    nc = bacc.Bacc(target_bir_lowering=False, num_devices=NCORES)
    xT_d = nc.declare_dram_parameter("xT", [D, TT], BF16, isOutput=False)
    wq_d = nc.declare_dram_parameter("wq", [128, D], BF16, isOutput=False)
    wk_d = nc.declare_dram_parameter("wk", [128, D], BF16, isOutput=False)
    wv_d = nc.declare_dram_parameter("wv", [128, D], BF16, isOutput=False)
    wo_d = nc.declare_dram_parameter("wo", [128, D], BF16, isOutput=False)
    tri2_d = nc.declare_dram_parameter("tri2", [128, 256], BF16, isOutput=False)
    ident_d = nc.declare_dram_parameter("ident", [128, 128], BF16, isOutput=False)
    ones32_d = nc.declare_dram_parameter("ones32", [128, 32], BF16, isOutput=False)
    out_d = nc.declare_dram_parameter("out", [TT, D], BF16, isOutput=True)

    with tile.TileContext(nc) as tc:
        with tc.tile_pool(name="consts", bufs=1) as consts, \
             tc.tile_pool(name="xin", bufs=24) as xin, \
             tc.tile_pool(name="qkvp", bufs=2) as qkvp, \
             tc.tile_pool(name="attnp", bufs=2) as attnp, \
             tc.tile_pool(name="outp", bufs=3) as outp, \
             tc.tile_pool(name="ps_mm", bufs=4, space="PSUM") as ps_mm, \
             tc.tile_pool(name="ps_st", bufs=2, space="PSUM") as ps_st:

            wq_sb = consts.tile([128, D], BF16)
            wk_sb = consts.tile([128, D], BF16)
            wv_sb = consts.tile([128, D], BF16)
            wo_sb = consts.tile([128, D], BF16)
            tri2_sb = consts.tile([128, 256], BF16)
            ident_sb = consts.tile([128, 128], BF16)
            ones32_sb = consts.tile([128, 32], BF16)
            # wq + the first x chunk gate the first matmul: issue them first.
            nc.sync.dma_start(wq_sb[:], wq_d[:, :])

            state = {}  # per-batch tiles

            def alloc_batch(b):
                state[b] = {
                    "qt": qkvp.tile([128, T], BF16, name=f"qt{b}", tag="qt"),
                    "kt": qkvp.tile([128, T], BF16, name=f"kt{b}", tag="kt"),
                    "vt": qkvp.tile([128, T], BF16, name=f"vt{b}", tag="vt"),
                    "vaug": None, "ctxT": None, "ctx": {}, "xts": {},
                }

            def load_chunk_x(b, tcn):
                """Issue the 8 x-tile DMAs for one 512-token chunk."""
                s = state[b]
                xts = []
                for ct in range(CT):
                    xt_t = xin.tile([128, 512], BF16,
                                    name=f"x{b}_{tcn}_{ct}", tag="xt")
                    nc.sync.dma_start(
                        xt_t[:],
                        xT_d[ct * 128:(ct + 1) * 128,
                             b * T + tcn * 512: b * T + (tcn + 1) * 512])
                    xts.append(xt_t)
                s["xts"][tcn] = xts

            def qkv_chunk_ops(b, tcn):
                """Emitter closures for one 512-token QKV chunk of batch b.

                The x DMAs for (b, tcn) must already be issued via
                load_chunk_x."""
                s = state[b]
                chunks = []
                for wsb, dst in ((wq_sb, "qt"), (wk_sb, "kt"), (wv_sb, "vt")):
                    def mk_group(wsb=wsb, dst=dst):
                        def f():
                            xts = s["xts"][tcn]
                            ps = ps_mm.tile([128, 512], F32, name="mmps",
                                            tag="mm")
                            for ct in range(CT):
                                nc.tensor.matmul(
                                    ps[:], wsb[:, ct * 128:(ct + 1) * 128],
                                    xts[ct][:], start=(ct == 0),
                                    stop=(ct == CT - 1))
                            nc.vector.tensor_copy(
                                s[dst][:, tcn * 512:(tcn + 1) * 512], ps[:])
                        return f
                    chunks.append([mk_group()])
                return chunks

            def vtrans_chunk_ops(b, c):
                """Transpose V chunk c (k-tiles 4c..4c+3) into vaug layout.

                vaug[:, k*130 + s*65 + 0:64] = V head s; col s*65+64 = 1.0
                """
                s = state[b]
                chunks = []
                ops = []
                if s["vaug"] is None:
                    s["vaug"] = qkvp.tile([128, NK * 130], BF16,
                                          name=f"vaug{b}", tag="vaug")

                    def ones_f():
                        va4 = s["vaug"][:].rearrange(
                            "p (k s c) -> p k s c", k=NK, s=2)
                        nc.vector.tensor_copy(
                            va4[:, :, :, 64:65],
                            ones32_sb[:].rearrange(
                                "p (k s c) -> p k s c", k=NK, s=2))
                    ops.append(ones_f)
                for kj in range(4 * c, 4 * c + 4):
                    def mk(kj=kj):
                        def f():
                            vps = ps_mm.tile([128, 128], BF16,
                                             name="vps", tag="mm")
                            nc.tensor.transpose(
                                vps[:], s["vt"][:, kj * 128:(kj + 1) * 128],
                                ident_sb[:])
                            nc.vector.tensor_copy(
                                s["vaug"][:, kj * 130: kj * 130 + 130].rearrange(
                                    "p (s c) -> p s c", s=2)[:, :, 0:64],
                                vps[:].rearrange("p (s c) -> p s c", s=2))
                        return f
                    ops.append(mk(kj))
                    if len(ops) == 2:
                        chunks.append(ops)
                        ops = []
                if ops:
                    chunks.append(ops)
                return chunks

            def proj_quarter_ops(b, qc):
                s = state[b]
                chunks = []
                ops = []
                for tt_i in range(qc * 4, qc * 4 + 4):
                    def mk(tt_i=tt_i):
                        def f():
                            osb = outp.tile([128, D], BF16, name="osb",
                                            tag="osb")
                            for oc in range(2):
                                op = ps_mm.tile([128, 512], F32,
                                                name="op", tag="mm")
                                nc.tensor.matmul(
                                    op[:],
                                    s["ctxT"][:, tt_i * 128:(tt_i + 1) * 128],
                                    wo_sb[:, oc * 512:(oc + 1) * 512],
                                    start=True, stop=True)
                                nc.vector.tensor_copy(
                                    osb[:, oc * 512:(oc + 1) * 512], op[:])
                                nc.sync.dma_start(
                                    out_d[b * T + tt_i * 128:
                                          b * T + (tt_i + 1) * 128,
                                          oc * 512:(oc + 1) * 512],
                                    osb[:, oc * 512:(oc + 1) * 512])
                        return f
                    ops.append(mk(tt_i))
                    if len(ops) == 2:
                        chunks.append(ops)
                        ops = []
                if ops:
                    chunks.append(ops)
                return chunks

            pend = []  # global score->PV pipeline, spills across sections

            def emit_pv():
                sec, ctx0, ctx1, vaug, kmax, kj, off, ptt = pend.pop(0)
                vb = kj * 130
                nc.tensor.matmul(
                    ctx0[:, off:512], vaug[:, vb: vb + 65], ptt[:, off:512],
                    start=(kj == 0), stop=(kj == kmax - 1),
                    skip_group_check=True)
                nc.tensor.matmul(
                    ctx1[:, off:512], vaug[:, vb + 65: vb + 130],
                    ptt[:, 512 + off:1024],
                    start=(kj == 0), stop=(kj == kmax - 1),
                    skip_group_check=True)

            def kj_stream(b, qc, filler, epi=None, last=False):
                """Scores+exp+mask for this section; PV pops trail by LOOK.

                The previous section's trailing PVs drain during our first
                iterations; `epi` (its normalization epilogue) fires as soon
                as they are done, and filler is held until then so PSUM "mm"
                slots can recycle.
                """
                s = state[b]
                if s["ctxT"] is None:
                    s["ctxT"] = qkvp.tile([128, T], BF16,
                                          name=f"ctxT{b}", tag="ctxT")
                sec = b * NQ + qc
                kmax = (qc + 1) * 4
                qlo = qc * 512
                ctx0 = ps_mm.tile([65, 512], F32, name="ctx0", tag="mm")
                ctx1 = ps_mm.tile([65, 512], F32, name="ctx1", tag="mm")
                s["ctx"][qc] = (ctx0, ctx1)
                qt, kt, vaug = s["qt"], s["kt"], s["vaug"]
                nfill = len(filler)
                done_f = 0
                epi1, epi2 = epi if epi is not None else (None, None)
                epi1_done = epi1 is None
                epi_done = epi is None
                epi2_at = None

                def pop_filler(upto):
                    nonlocal done_f
                    while done_f < upto:
                        for f in filler[done_f]:
                            f()
                        done_f += 1
                for kj in range(kmax):
                    off = max(0, kj * 128 - qlo)
                    ksl = slice(kj * 128, (kj + 1) * 128)
                    st = ps_st.tile([128, 1024], F32, name="st", tag="st")
                    nc.tensor.matmul(
                        st[:, off:512], kt[0:64, ksl],
                        qt[0:64, qlo + off: qlo + 512],
                        start=True, stop=True)
                    nc.tensor.matmul(
                        st[:, 512 + off:1024], kt[64:128, ksl],
                        qt[64:128, qlo + off: qlo + 512],
                        start=True, stop=True)
                    ptt = attnp.tile([128, 1024], BF16, name="pt",
                                     tag="pt", bufs=9)
                    nc.scalar.activation(
                        ptt[:, off:1024], st[:, off:1024], EXP, scale=0.125)
                    if kj * 128 >= qlo:  # diagonal: causal triangle mask
                        pv = ptt[:].rearrange("p (s c) -> p s c", s=2)
                        nc.gpsimd.tensor_mul(
                            pv[:, :, off: off + 128],
                            pv[:, :, off: off + 128],
                            tri2_sb[:].rearrange("p (s c) -> p s c", s=2))
                    pend.append((sec, ctx0, ctx1, vaug, kmax, kj, off, ptt))
                    if len(pend) > LOOK:
                        emit_pv()
                    # drain the previous section's PVs at double rate
                    while pend and pend[0][0] != sec and len(pend) > 2:
                        emit_pv()
                    if not epi1_done and not (pend and pend[0][0] != sec):
                        epi1()  # start the DVE reciprocal chain
                        epi1_done = True
                        epi2_at = kj + 2
                    if not epi_done and epi2_at is not None and kj >= epi2_at:
                        epi2()  # broadcast + mul: releases ctx slots
                        epi_done = True
                    if epi_done:
                        # burst filler (>=3 chunks ~5us dense PE): HAM warmth
                        want = nfill * (kj + 1) // kmax
                        if want - done_f >= 3 or kj >= kmax - 1:
                            pop_filler(want)
                if not epi_done:
                    while pend and pend[0][0] != sec:
                        emit_pv()
                    if not epi1_done:
                        epi1()
                    epi2()
                    epi_done = True
                pop_filler(nfill)
                if last:
                    while pend:
                        emit_pv()

            def epilogue_pre(b, qc):
                """DVE-only part of the deferred normalization: 1/colsum."""
                s = state[b]
                recs = []
                for h, ctx in zip((0, 1), s["ctx"][qc]):
                    deni = attnp.tile([1, 512], F32, name="deni", tag="deni")
                    nc.vector.tensor_copy(deni[:], ctx[64:65, :])
                    rec = attnp.tile([1, 512], F32, name="rec", tag="rec")
                    nc.vector.reciprocal_approx_fast(rec[:], deni[:])
                    recs.append(rec)
                return recs

            def epilogue_ops(b, qc, recs):
                """GpSimd broadcast + DVE multiply; releases ctx PSUM slots."""
                s = state[b]
                ctxs = s["ctx"].pop(qc)
                qlo = qc * 512
                ops = []
                for h in (0, 1):
                    def mk(h=h, ctx=ctxs[h], rec=recs[h]):
                        def f():
                            bcs = attnp.tile([64, 512], F32, name="bcs",
                                             tag="bcs")
                            nc.gpsimd.partition_broadcast(bcs[:], rec[:])
                            nc.vector.tensor_mul(
                                s["ctxT"][h * 64:(h + 1) * 64, qlo: qlo + 512],
                                ctx[0:64, :], bcs[:])
                        return f
                    ops.append(mk(h))
                return [ops]

            # ---- prologue: first QKV chunk + its V-transposes ----
            alloc_batch(0)
            load_chunk_x(0, 0)
            # remaining consts, ordered by first use
            nc.sync.dma_start(wk_sb[:], wk_d[:, :])
            nc.sync.dma_start(wv_sb[:], wv_d[:, :])
            nc.sync.dma_start(ident_sb[:], ident_d[:, :])
            nc.sync.dma_start(tri2_sb[:], tri2_d[:, :])
            nc.sync.dma_start(ones32_sb[:], ones32_d[:, :])
            nc.sync.dma_start(wo_sb[:], wo_d[:, :])
            load_chunk_x(0, 1)  # prefetch: consumed as section-0 filler
            for ch in qkv_chunk_ops(0, 0):
                for f in ch:
                    f()
            for ch in vtrans_chunk_ops(0, 0):
                for f in ch:
                    f()

            # ---- pipelined main loop with a global just-in-time QKV queue:
            # section i emits QKV chunk i+1 (flattened over (batch, tcn)),
            # which is exactly what section i+1's attention stream consumes.
            # x DMAs for chunk i+2 are issued at the START of section i+1 so
            # the data is resident long before its compute pops as filler.
            prev = None  # (b, qc) awaiting epilogue
            for b in range(B):
                for qc in range(NQ):
                    sec = b * NQ + qc
                    nxt = sec + 1  # global chunk index to emit now
                    filler = []
                    if prev is not None:
                        box = {}

                        def epi1(prev=prev, box=box):
                            box["recs"] = epilogue_pre(*prev)

                        def epi2(prev=prev, box=box):
                            for ch in epilogue_ops(*prev, box["recs"]):
                                for f in ch:
                                    f()
                        epi = (epi1, epi2)
                    else:
                        epi = None
                    if nxt < B * NQ:
                        nb, ntc = divmod(nxt, NQ)
                        if nb not in state:
                            alloc_batch(nb)
                        if nxt + 1 < B * NQ:
                            pb, ptc = divmod(nxt + 1, NQ)
                            if pb not in state:
                                alloc_batch(pb)
                            load_chunk_x(pb, ptc)
                        filler += qkv_chunk_ops(nb, ntc)
                        filler += vtrans_chunk_ops(nb, ntc)
                    if b >= 1:
                        filler += proj_quarter_ops(b - 1, qc)
                    if b == B - 1 and qc >= 1:
                        filler += proj_quarter_ops(b, qc - 1)
                    kj_stream(b, qc, filler, epi=epi,
                              last=(b == B - 1 and qc == NQ - 1))
                    prev = (b, qc)
            recs = epilogue_pre(*prev)
            for ch in epilogue_ops(*prev, recs):
                for f in ch:
                    f()
            for ch in proj_quarter_ops(B - 1, 3):
                for f in ch:
                    f()

    nc.compile()
    return nc


def make_in_maps(x, Wq, Wk, Wv, Wo):
    bf = ml_dtypes.bfloat16
    xT = np.ascontiguousarray(x.reshape(TT, D).T).astype(bf)
    tri = np.triu(np.ones((128, 128), np.float32))  # keep k<=q
    tri2 = np.concatenate([tri, tri], axis=1).astype(bf)
    ident = np.eye(128, dtype=np.float32).astype(bf)
    ones32 = np.ones((128, 32), bf)
    in_maps = []
    for c in range(NCORES):
        dsl = slice(c * 128, (c + 1) * 128)
        wq = np.concatenate(
            [Wq[dsl, ct * 128:(ct + 1) * 128].T for ct in range(CT)], axis=1)
        wk = np.concatenate(
            [Wk[dsl, ct * 128:(ct + 1) * 128].T for ct in range(CT)], axis=1)
        wv = np.concatenate(
            [Wv[dsl, ct * 128:(ct + 1) * 128].T for ct in range(CT)], axis=1)
        wo = Wo[:, dsl].T
        in_maps.append({
            "xT": xT,
            "wq": np.ascontiguousarray(wq).astype(bf),
            "wk": np.ascontiguousarray(wk).astype(bf),
            "wv": np.ascontiguousarray(wv).astype(bf),
            "wo": np.ascontiguousarray(wo).astype(bf),
            "tri2": tri2, "ident": ident, "ones32": ones32,
        })
    return in_maps


_NC_CACHE = None


def kernel_run(x, Wq, Wk, Wv, Wo, bo, trace=False, trace_cores=None):
    global _NC_CACHE
    if _NC_CACHE is None:
        _NC_CACHE = build_nc()
    nc = _NC_CACHE
    in_maps = make_in_maps(np.asarray(x), np.asarray(Wq), np.asarray(Wk),
                           np.asarray(Wv), np.asarray(Wo))
    res = None
    for attempt in range(3):
        try:
            res = run_bass_kernel_spmd(nc, in_maps,
                                       core_ids=list(range(NCORES)),
                                       trace=trace, trace_cores=trace_cores)
            break
        except Exception:
            if attempt == 2:
                raise
            import time as _time
            _time.sleep(5)
    acc = res.results[0]["out"].astype(np.float32)
    for c in range(1, NCORES):
        acc += res.results[c]["out"].astype(np.float32)
    outv = (acc + np.asarray(bo, dtype=np.float32)).astype(np.float32)
    return outv.reshape(B, T, D), res


def kernel(x, Wq, Wk, Wv, Wo, bo):
    out, _ = kernel_run(x, Wq, Wk, Wv, Wo, bo)
    return out
